# revision 1
# baseline (speedup 1.0000x reference)
"""MDN-RNN mixture-density loss kernel for Trainium2, SPMD over 8 NeuronCores.

Math (per token row i):
    means/logstds: [K, D] slices of s_mean/s_logstd rows
    z      = (target - mean_k) * exp(-logstd_k)
    logp_k = -0.5 * sum_d z^2 - sum_d logstd_k
    loss   = -mean_i logsumexp_k(log_mix_coeffs + logp_k)

Sharding: data-parallel on the token dim N=16384 -> 2048 rows per core,
no cross-device communication; each core emits a [128,1] partial sum of
per-row -logsumexp values (partition p holds the sum over its 16 rows),
combined into the scalar mean on the host.

Engine split per 128-row tile (fp32, rows on partitions):
    DVE:    part of sls_k = sum_d logstd (grouped 3D reduce),
            diff = target(bcast over k) - mean, z = diff*e1 (3D mult),
            logsumexp smalls (scalar_tensor_tensor / reduce-max)
    ACT:    rest of sls_k (Copy w/ accumulate), e1 = exp(-logstd) in-place,
            per-k h_k = sum(z^2) via Square w/ accumulate, logsumexp exp

All per-tile scalar-engine functions (Copy/Exp/Square) live in one ACT
table set; Ln is deferred to a single [128,T] pass after the loop so the
compiler does not ping-pong activation table loads inside the loop.
"""

import sys

if "/opt/trn_rl_repo" not in sys.path:
    sys.path.insert(0, "/opt/trn_rl_repo")

import numpy as np

N = 16384
K = 5
D = 1088
KD = K * D
NCORES = 8
R = N // NCORES          # 2048 rows per core
P = 128                  # partitions
T = R // P               # 16 tiles per core

# number of per-k sum(logstd) reductions on ACT (rest grouped on DVE)
SLS_ACT_K = 2

_NC = None


def _build():
    import concourse.bacc as bacc
    import concourse.bass as bass
    import concourse.tile as tile
    from concourse import mybir

    AF = mybir.ActivationFunctionType
    AL = mybir.AluOpType
    AX = mybir.AxisListType
    f32 = mybir.dt.float32

    nc = bacc.Bacc("TRN2", debug=False)
    tgt = nc.dram_tensor("tgt", [R, D], f32, kind="ExternalInput").ap()
    mean = nc.dram_tensor("mean", [R, KD], f32, kind="ExternalInput").ap()
    lstd = nc.dram_tensor("lstd", [R, KD], f32, kind="ExternalInput").ap()
    lmx = nc.dram_tensor("lmx", [P, T * K], f32, kind="ExternalInput").ap()
    out = nc.dram_tensor("partial", [P, 1], f32, kind="ExternalOutput").ap()

    with tile.TileContext(nc) as tc:
        with (
            tc.tile_pool(name="tgt_p", bufs=3) as tgt_p,
            tc.tile_pool(name="mean_p", bufs=3) as mean_p,
            tc.tile_pool(name="lstd_p", bufs=3) as lstd_p,
            tc.tile_pool(name="e1_p", bufs=2) as e1_p,
            tc.tile_pool(name="small_p", bufs=3) as small_p,
            tc.tile_pool(name="persist", bufs=1) as persist,
        ):
            t_lmx = persist.tile([P, T * K], f32)
            nc.sync.dma_start(out=t_lmx, in_=lmx)
            t_nmacc = persist.tile([P, T], f32)   # per-tile -max_k score
            t_sacc = persist.tile([P, T], f32)    # per-tile sum_k exp(score+nm)

            state = {}

            def emit_a(t):
                """Front stage: DMAs, sum(logstd), e1 = exp(-logstd), diff."""
                rows = slice(t * P, (t + 1) * P)
                split = t == T - 1 or t == 0
                t_tgt = tgt_p.tile([P, D], f32)
                t_mean = mean_p.tile([P, K, D], f32)
                t_lstd = lstd_p.tile([P, K, D], f32)
                mean3 = mean[rows].rearrange("p (k d) -> p k d", k=K)
                lstd3 = lstd[rows].rearrange("p (k d) -> p k d", k=K)
                if not split:
                    nc.sync.dma_start(out=t_lstd, in_=lstd3)
                    nc.sync.dma_start(out=t_tgt, in_=tgt[rows])
                    nc.sync.dma_start(out=t_mean, in_=mean3)
                else:
                    # chunked so first compute starts after ~1/5 of the load
                    nc.sync.dma_start(out=t_lstd[:, 0, :], in_=lstd3[:, 0, :])
                    nc.sync.dma_start(out=t_tgt, in_=tgt[rows])
                    for k in range(1, K):
                        nc.sync.dma_start(out=t_lstd[:, k, :], in_=lstd3[:, k, :])
                    for k in range(K):
                        nc.sync.dma_start(out=t_mean[:, k, :], in_=mean3[:, k, :])

                t_sls = small_p.tile([P, K], f32)
                for k in range(SLS_ACT_K):
                    nc.scalar.activation(
                        out=t_lstd[:, k, :], in_=t_lstd[:, k, :], func=AF.Copy,
                        accum_out=t_sls[:, k : k + 1],
                    )
                if SLS_ACT_K < K:
                    nc.vector.tensor_reduce(
                        out=t_sls[:, SLS_ACT_K:K], in_=t_lstd[:, SLS_ACT_K:K, :],
                        axis=AX.X, op=AL.add,
                    )

                # e1 = exp(-logstd) into its own tile: no WAR against the
                # sls reads of lstd, so ACT can run it whenever lstd lands
                t_e1 = e1_p.tile([P, K, D], f32)
                nc.scalar.activation(out=t_e1, in_=t_lstd, func=AF.Exp, scale=-1.0)

                # diff = target (broadcast over k) - mean, in place (DVE)
                tgt_b = bass.AP(
                    tensor=t_tgt.tensor, offset=t_tgt.offset,
                    ap=[t_tgt.ap[0], [0, K], t_tgt.ap[1]],
                )
                if not split:
                    nc.vector.tensor_tensor(out=t_mean, in0=tgt_b, in1=t_mean, op=AL.subtract)
                else:
                    for k in range(K):
                        nc.vector.tensor_tensor(
                            out=t_mean[:, k, :], in0=t_tgt, in1=t_mean[:, k, :],
                            op=AL.subtract,
                        )
                state[t] = (t_mean, t_e1, t_sls)

            def emit_b(t):
                """Back stage: z, squares w/ accumulate, logsumexp smalls."""
                split = t == T - 1 or t == 0
                t_mean, t_e1, t_sls = state.pop(t)
                t_h = small_p.tile([P, K], f32)
                if not split:
                    # z = diff * e1 (3D DVE mult, in place on the mean tile)
                    nc.vector.tensor_tensor(out=t_mean, in0=t_mean, in1=t_e1, op=AL.mult)
                    # h_k = sum_d z^2 via ACT Square w/ accumulate (in place)
                    for k in range(K):
                        nc.scalar.activation(
                            out=t_mean[:, k, :], in_=t_mean[:, k, :], func=AF.Square,
                            accum_out=t_h[:, k : k + 1],
                        )
                else:
                    for k in range(K):
                        nc.vector.tensor_tensor(
                            out=t_mean[:, k, :], in0=t_mean[:, k, :],
                            in1=t_e1[:, k, :], op=AL.mult,
                        )
                        nc.scalar.activation(
                            out=t_mean[:, k, :], in_=t_mean[:, k, :], func=AF.Square,
                            accum_out=t_h[:, k : k + 1],
                        )

                # score_k = -0.5*h_k - sls_k + lmx_k ; nm = -max_k score
                t_q = small_p.tile([P, K], f32)
                nc.vector.scalar_tensor_tensor(
                    out=t_q, in0=t_h, scalar=-0.5, in1=t_sls,
                    op0=AL.mult, op1=AL.subtract,
                )
                t_c = small_p.tile([P, K], f32)
                nc.vector.tensor_tensor(
                    out=t_c, in0=t_q, in1=t_lmx[:, t * K : (t + 1) * K], op=AL.add
                )
                nc.vector.tensor_reduce(
                    out=t_nmacc[:, t : t + 1], in_=t_c, axis=AX.X, op=AL.max, negate=True
                )
                # S_t = sum_k exp(score + nm)
                t_e = small_p.tile([P, K], f32)
                nc.scalar.activation(
                    out=t_e, in_=t_c, func=AF.Exp, bias=t_nmacc[:, t : t + 1],
                    scale=1.0, accum_out=t_sacc[:, t : t + 1],
                )

            # software-pipelined emission: tile t+1's front stage is queued
            # before tile t's back stage so neither engine head-of-line
            # blocks on the cross-engine z/square seam
            emit_a(0)
            for t in range(T):
                if t + 1 < T:
                    emit_a(t + 1)
                emit_b(t)

            # loss rows: nm_t - ln(S_t); single Ln pass keeps Ln out of the loop
            t_lns = persist.tile([P, T], f32)
            nc.scalar.activation(out=t_lns, in_=t_sacc, func=AF.Ln)
            t_accv = persist.tile([P, T], f32)
            nc.vector.tensor_tensor(out=t_accv, in0=t_nmacc, in1=t_lns, op=AL.subtract)
            t_tot = persist.tile([P, 1], f32)
            nc.vector.tensor_reduce(out=t_tot, in_=t_accv, axis=AX.X, op=AL.add)
            nc.sync.dma_start(out=out, in_=t_tot)

    nc.compile()
    return nc


def get_nc():
    global _NC
    if _NC is None:
        _NC = _build()
    return _NC


def make_in_maps(target, s_mean, s_logstd, log_mix_coeffs):
    target = np.ascontiguousarray(np.asarray(target, dtype=np.float32))
    s_mean = np.ascontiguousarray(np.asarray(s_mean, dtype=np.float32))
    s_logstd = np.ascontiguousarray(np.asarray(s_logstd, dtype=np.float32))
    lm = np.ascontiguousarray(np.asarray(log_mix_coeffs, dtype=np.float32))
    in_maps = []
    for c in range(NCORES):
        rows = slice(c * R, (c + 1) * R)
        # pack log-mix so tile t's [128, K] block sits at columns [t*K, (t+1)*K)
        lmx = lm[rows].reshape(T, P, K).transpose(1, 0, 2).reshape(P, T * K)
        in_maps.append({
            "tgt": np.ascontiguousarray(target[rows]),
            "mean": np.ascontiguousarray(s_mean[rows]),
            "lstd": np.ascontiguousarray(s_logstd[rows]),
            "lmx": np.ascontiguousarray(lmx),
        })
    return in_maps


def combine(results):
    total = sum(float(np.asarray(r["partial"], dtype=np.float64).sum()) for r in results)
    return np.float32(total / N)


def kernel(target, s_mean, s_logstd, log_mix_coeffs):
    from concourse.bass_utils import run_bass_kernel_spmd

    nc = get_nc()
    in_maps = make_in_maps(target, s_mean, s_logstd, log_mix_coeffs)
    res = run_bass_kernel_spmd(nc, in_maps, core_ids=list(range(NCORES)))
    return combine(res.results)

